# revision 2
# baseline (speedup 1.0000x reference)
"""CosineSimilarityAttention Trainium2 kernel (8 NeuronCores, SPMD).

Sharding: token-parallel. Global tokens = 2 batches x 4096. Core c handles
batch (c // 4), query rows (c % 4)*1024 .. +1024. Each core computes K/V
projections for its whole batch (4096 tokens) -- replicated within each
4-core batch group -- plus Q for its own 1024 tokens, then attention and
the output projection for its token slice. Outputs concatenate on host.

Math per batch (faithful to reference):
  qkv = x @ w_qkv.T ; split q,k,v ; reshape heads h=12, dh=64
  q *= 1/sqrt(||q||_heads + eps)   (L2 norm over the HEADS axis, per (n, dh))
  k *= 1/sqrt(||k||_heads + eps)
  out_h = softmax((q_h k_h^T) / scale_h) v_h   (no max-subtract: |logits|<~2)
  y = concat_h(out_h) @ w_out.T + b_out
"""

import numpy as np

import concourse.bass as bass
import concourse.mybir as mybir
import concourse.tile as tile
from concourse.bass_utils import run_bass_kernel_spmd
from concourse.masks import make_identity

F32 = mybir.dt.float32
BF16 = mybir.dt.bfloat16

B = 2
N = 4096          # tokens per batch
D = 768           # model dim
H = 12            # heads
DH = 64           # head dim
INNER = H * DH    # 768
EPS = 1e-8
NQ = 1024         # query tokens per core
NCORES = 8
BLK = 512         # projection token block
KT = N // 128     # 32 key tiles of 128


def _split_multi_waits(nc):
    """This container's walrus accepts only ONE sync-wait per instruction.
    Hoist extra waits into standalone EVSEM instructions placed just before."""
    n = 0
    for f in nc.m.functions:
        for bb in f.blocks:
            insts = list(bb.instructions)
            out = []
            for inst in insts:
                si = inst.sync_info
                if si is not None and si.on_wait is not None and len(si.on_wait) > 1:
                    waits = list(si.on_wait)
                    for j, w in enumerate(waits[:-1]):
                        ev = mybir.InstEventSemaphore(
                            name=f"{inst.name}-evw{j}",
                            engine=inst.engine,
                            sync_info=mybir.SyncInfo(on_wait=[w], on_update=[]),
                        )
                        out.append(ev)
                        n += 1
                    si.on_wait = [waits[-1]]
                out.append(inst)
            bb.instructions = out
    return n


def _build_program(inv_scale):
    """Build the single SPMD Bass program. inv_scale: list of 12 floats."""
    nc = bass.Bass()
    xb = nc.declare_dram_parameter("xb", [N, D], F32, isOutput=False)
    qx = nc.declare_dram_parameter("qx", [NQ, D], F32, isOutput=False)
    wqkvT = nc.declare_dram_parameter("wqkvT", [D, 3 * INNER], F32, isOutput=False)
    woT = nc.declare_dram_parameter("woT", [INNER, D], F32, isOutput=False)
    bout = nc.declare_dram_parameter("bout", [1, D], F32, isOutput=False)
    selin = nc.declare_dram_parameter("selin", [128, 128], F32, isOutput=False)
    y = nc.declare_dram_parameter("y", [NQ, D], F32, isOutput=True)

    with tile.TileContext(nc) as tc:
        with tc.tile_pool(name="const", bufs=1) as constp, \
             tc.tile_pool(name="persist", bufs=1) as persist:
            # --- constants ---
            ident = constp.tile([128, 128], F32)
            make_identity(nc, ident)
            sel_st = constp.tile([128, 128], F32)
            nc.sync.dma_start(out=sel_st, in_=selin[:, :])
            sel_bf = constp.tile([128, 128], BF16)
            nc.vector.tensor_copy(sel_bf, sel_st)
            ones_f = constp.tile([1, 64], F32)
            nc.vector.memset(ones_f, 1.0)
            ones_bf = constp.tile([1, 128], BF16)
            nc.vector.memset(ones_bf, 1.0)
            eps_t = constp.tile([128, 1], F32)
            nc.vector.memset(eps_t, EPS)
            invs = constp.tile([128, 6], F32)
            for dt in range(6):
                nc.vector.memset(invs[0:64, dt:dt + 1], float(inv_scale[2 * dt]))
                nc.vector.memset(invs[64:128, dt:dt + 1], float(inv_scale[2 * dt + 1]))
            b_st = constp.tile([1, D], F32)
            nc.sync.dma_start(out=b_st, in_=bout[:, :])
            b_bf = constp.tile([1, D], BF16)
            nc.vector.tensor_copy(b_bf, b_st)

            # --- persistent activations ---
            khat = persist.tile([128, 6, N], BF16)     # k^T normalized  [dim, tok]
            qhat = persist.tile([128, H, NQ], BF16)    # q^T per head, K=128 zero-padded
            vhat = persist.tile([128, KT, H * 65], BF16)  # v [tok, h*65] (+ones col)

            # ones columns of vhat (col 64 of every 65-block)
            vones = vhat.rearrange("p t (h c) -> p t h c", c=65)[:, :, :, 64:65]
            nc.vector.memset(vones, 1.0)
            nc.vector.memset(qhat, 0.0)

            # ---------------- phase W+P: weights, projections, head-norm ----
            with tc.tile_pool(name="pw", bufs=1) as pwp:
              wq = pwp.tile([128, 6, 3 * INNER], BF16)
              with tc.tile_pool(name="wstage", bufs=1) as wst:
                for dt in range(6):
                    st = wst.tile([128, 3 * INNER], F32, tag="wst")
                    nc.sync.dma_start(out=st, in_=wqkvT[dt * 128:(dt + 1) * 128, :])
                    nc.vector.tensor_copy(wq[:, dt, :], st)
              with tc.tile_pool(name="pstage", bufs=1) as pstage, \
                   tc.tile_pool(name="pxT", bufs=2) as pxT, \
                   tc.tile_pool(name="pkf", bufs=2) as pkf, \
                   tc.tile_pool(name="psmall", bufs=1) as psmall, \
                   tc.tile_pool(name="pksq", bufs=1) as pksq, \
                   tc.tile_pool(name="psumA", bufs=4, space="PSUM") as pA, \
                   tc.tile_pool(name="psumB", bufs=2, space="PSUM") as pB:

                  def proj_block(src, blk_i, is_q):
                      # load + transpose x block [512, D] -> xT [dim, tok] bf16
                      xst = pstage.tile([128, 4, D], F32, tag="xst")
                      nc.sync.dma_start(
                          out=xst,
                          in_=src[blk_i * BLK:(blk_i + 1) * BLK, :].rearrange(
                              "(t p) d -> p t d", p=128),
                      )
                      xT = pxT.tile([128, 6, BLK], BF16, tag="xT")
                      for dt in range(6):
                          tp = pA.tile([128, 512], F32, tag="pA")
                          for tt in range(4):
                              nc.tensor.transpose(
                                  tp[:, tt * 128:(tt + 1) * 128],
                                  xst[:, tt, dt * 128:(dt + 1) * 128], ident)
                          nc.vector.tensor_copy(xT[:, dt, :], tp)

                      wbase = 0 if is_q else INNER
                      # q^T / k^T projection [dim_out, tok]
                      kf = pkf.tile([128, 6, BLK], BF16, tag="kf")
                      for dt in range(6):
                          kp = pA.tile([128, 512], F32, tag="pA")
                          for ks in range(6):
                              nc.tensor.matmul(
                                  kp,
                                  wq[:, ks, wbase + dt * 128: wbase + (dt + 1) * 128],
                                  xT[:, ks, :],
                                  start=(ks == 0), stop=(ks == 5))
                          nc.vector.tensor_copy(kf[:, dt, :], kp)
                      # ssq over heads: sel matmul on squares
                      sq = pA.tile([128, 512], F32, tag="pA")
                      for dt in range(6):
                          ksq = pksq.tile([128, BLK], BF16, tag="ksq")
                          nc.vector.tensor_mul(ksq, kf[:, dt, :], kf[:, dt, :])
                          nc.tensor.matmul(sq, sel_bf, ksq,
                                           start=(dt == 0), stop=(dt == 5))
                      nrm = psmall.tile([128, BLK], F32, tag="nrm")
                      nc.scalar.activation(nrm, sq, mybir.ActivationFunctionType.Sqrt)
                      u = psmall.tile([128, BLK], F32, tag="u")
                      nc.scalar.activation(u, nrm, mybir.ActivationFunctionType.Sqrt,
                                           bias=eps_t[:, :])
                      rq = psmall.tile([128, BLK], F32, tag="rq")
                      nc.vector.reciprocal(rq, u)
                      bsl = bass.ts(blk_i, BLK)
                      if is_q:
                          # zero-padded per-head layout: head 2dt on rows 0:64,
                          # head 2dt+1 on rows 64:128, other rows stay zero.
                          # 1/scale_h is folded into qhat so exp needs no scale.
                          for dt in range(6):
                              a = qhat[0:64, 2 * dt, bsl]
                              b = qhat[64:128, 2 * dt + 1, bsl]
                              nc.vector.tensor_mul(a, kf[0:64, dt, :], rq[0:64, :])
                              nc.vector.tensor_mul(b, kf[64:128, dt, :],
                                                   rq[64:128, :])
                              nc.vector.tensor_scalar_mul(a, a,
                                                          invs[0:64, dt:dt + 1])
                              nc.vector.tensor_scalar_mul(b, b,
                                                          invs[64:128, dt:dt + 1])
                      else:
                          for dt in range(6):
                              nc.vector.tensor_mul(
                                  khat[:, dt, bsl], kf[:, dt, :], rq)
                      if is_q:
                          return
                      # v projection [tok, inner] -> vhat strided 65
                      for tt in range(4):
                          vp = pB.tile([128, 1024], F32, tag="pB")
                          for ks in range(6):
                              nc.tensor.matmul(vp[:, 0:512],
                                               xT[:, ks, tt * 128:(tt + 1) * 128],
                                               wq[:, ks, 2 * INNER:2 * INNER + 512],
                                               start=(ks == 0), stop=(ks == 5))
                              nc.tensor.matmul(vp[:, 512:768],
                                               xT[:, ks, tt * 128:(tt + 1) * 128],
                                               wq[:, ks, 2 * INNER + 512:3 * INNER],
                                               start=(ks == 0), stop=(ks == 5))
                          vdst = vhat[:, blk_i * 4 + tt, :].rearrange(
                              "p (h c) -> p h c", c=65)[:, :, 0:64]
                          nc.vector.tensor_copy(
                              vdst, vp[:, 0:768].rearrange("p (h c) -> p h c", c=64))

                  for blk in range(NQ // BLK):
                      proj_block(qx, blk, True)
                  for blk in range(N // BLK):
                      proj_block(xb, blk, False)

            # ---------------- phase A: attention ----------------
            with tc.tile_pool(name="opersist", bufs=1) as operp:
              oh_all = operp.tile([64, H, NQ], BF16)
              wo12 = operp.tile([64, H, D], BF16)
              with tc.tile_pool(name="wostage", bufs=2) as wost:
                for h in range(H):
                    wst_t = wost.tile([64, D], F32, tag="wost")
                    nc.sync.dma_start(out=wst_t, in_=woT[h * 64:(h + 1) * 64, :])
                    nc.vector.tensor_copy(wo12[:, h, :], wst_t)
              with tc.tile_pool(name="pP", bufs=6) as pP, \
                   tc.tile_pool(name="poraw", bufs=6) as poraw, \
                   tc.tile_pool(name="princ", bufs=2) as princ, \
                   tc.tile_pool(name="psumS", bufs=2, space="PSUM") as pS, \
                   tc.tile_pool(name="psumO", bufs=4, space="PSUM") as pO:
                  # head-pair processing: heads (2i, 2i+1) live on PE row
                  # groups 0-63 / 64-127 and run concurrently. Queries are
                  # split in 512-halves so every PSUM tile is one bank.
                  for hp in range(6):
                      hs = (2 * hp, 2 * hp + 1)
                      ots = {}
                      for h in hs:
                          for qh in range(2):
                              ot = pO.tile([65, 512], F32, tag="pO",
                                           name=f"ot_{h}_{qh}")
                              ots[(h, qh)] = ot
                      for kb in range(KT):
                          for qh in range(2):
                              qsl = bass.ts(qh, 512)
                              st = pS.tile([128, 1024], F32, tag="pS",
                                           name=f"st_{qh}")
                              for j, h in enumerate(hs):
                                  nc.tensor.matmul(
                                      st[:, j * 512:(j + 1) * 512],
                                      khat[:, hp, kb * 128:(kb + 1) * 128],
                                      qhat[:, h, qsl],
                                      start=True, stop=True)
                              pt = pP.tile([128, 1024], BF16, tag="pP",
                                           name=f"pt_{qh}")
                              nc.scalar.activation(
                                  pt, st, mybir.ActivationFunctionType.Exp)
                              for j, h in enumerate(hs):
                                  nc.tensor.matmul(
                                      ots[(h, qh)],
                                      vhat[:, kb, h * 65:(h + 1) * 65],
                                      pt[:, j * 512:(j + 1) * 512],
                                      start=(kb == 0), stop=(kb == KT - 1))
                      for h in hs:
                          for qh in range(2):
                              qsl = bass.ts(qh, 512)
                              o_raw = poraw.tile([65, 512], F32, tag="oraw",
                                                 name=f"oraw_{h}_{qh}")
                              if h % 2 == 0:
                                  nc.vector.tensor_copy(o_raw, ots[(h, qh)])
                              else:
                                  nc.scalar.copy(o_raw, ots[(h, qh)])
                              rinv = princ.tile([1, 512], F32, tag="rinv",
                                                name=f"rinv_{h}_{qh}")
                              nc.vector.reciprocal(rinv, o_raw[64:65, :])
                              rbc = pS.tile([128, 512], F32, tag="pS",
                                            name=f"rbc_{h}_{qh}")
                              nc.tensor.matmul(rbc[0:64, :], ones_f, rinv,
                                               start=True, stop=True)
                              nc.vector.tensor_mul(oh_all[:, h, qsl],
                                                   o_raw[0:64, :], rbc[0:64, :])

            # ---------------- phase Y: output projection ----------------
              with tc.tile_pool(name="pys", bufs=2) as pys, \
                   tc.tile_pool(name="psumY", bufs=2, space="PSUM") as pY:
                  for mt in range(NQ // 128):
                      yp = pY.tile([128, 1024], F32, tag="pY")
                      for h in range(H):
                          lhsT = oh_all[:, h, mt * 128:(mt + 1) * 128]
                          nc.tensor.matmul(yp[:, 0:512], lhsT, wo12[:, h, 0:512],
                                           start=(h == 0), stop=False)
                          nc.tensor.matmul(yp[:, 512:768], lhsT, wo12[:, h, 512:768],
                                           start=(h == 0), stop=False)
                      nc.tensor.matmul(yp[:, 0:512], ones_bf, b_bf[:, 0:512],
                                       start=False, stop=True)
                      nc.tensor.matmul(yp[:, 512:768], ones_bf, b_bf[:, 512:768],
                                       start=False, stop=True)
                      ys = pys.tile([128, D], F32, tag="ys")
                      nc.vector.tensor_copy(ys, yp[:, 0:768])
                      nc.sync.dma_start(out=y[mt * 128:(mt + 1) * 128, :], in_=ys)

    _split_multi_waits(nc)
    return nc


_prog_cache = {}


def _make_in_maps(inputs):
    x = np.ascontiguousarray(np.asarray(inputs["x"], dtype=np.float32))
    w_qkv = np.asarray(inputs["w_qkv"], dtype=np.float32)
    w_out = np.asarray(inputs["w_out"], dtype=np.float32)
    b_out = np.asarray(inputs["b_out"], dtype=np.float32).reshape(1, D)

    wqkvT = np.ascontiguousarray(w_qkv.T)            # [768, 2304]
    woT = np.ascontiguousarray(w_out.T)              # [768, 768]
    p = np.arange(128)
    sel = (p[:, None] % 64 == p[None, :] % 64).astype(np.float32)

    in_maps = []
    for c in range(NCORES):
        bi, qi = c // 4, c % 4
        in_maps.append({
            "xb": x[bi],
            "qx": np.ascontiguousarray(x[bi, qi * NQ:(qi + 1) * NQ]),
            "wqkvT": wqkvT,
            "woT": woT,
            "bout": b_out,
            "selin": sel,
        })
    return in_maps


def kernel(x, w_qkv, w_out, b_out, scale):
    scale = np.asarray(scale, dtype=np.float32)
    inv_scale = tuple(float(1.0 / s) for s in scale)
    nc = _prog_cache.get(inv_scale)
    if nc is None:
        nc = _build_program(inv_scale)
        _prog_cache[inv_scale] = nc

    in_maps = _make_in_maps(
        {"x": x, "w_qkv": w_qkv, "w_out": w_out, "b_out": b_out})

    res = run_bass_kernel_spmd(nc, in_maps, core_ids=list(range(NCORES)))
    out = np.empty((B, N, D), dtype=np.float32)
    for c in range(NCORES):
        bi, qi = c // 4, c % 4
        out[bi, qi * NQ:(qi + 1) * NQ] = res.results[c]["y"]
    return out

